# revision 1
# baseline (speedup 1.0000x reference)
"""CAAN kernel for Trainium2, 8-core data-parallel (one batch row per core).

Math: the reference is
    Q = R Wq^T + bq ; K = R Wk^T + bk ; V = R Wv^T + bv
    E = exp(Q K^T / sqrt(512)) ; saat = E / rowsum(E)
    winner = (saat V) W1^T W2^T + (W2 b1 + b2)

Two algebraic collapses make most of the network disappear:

1. The W1/W2 head is linear, so with c = W1^T W2[0]:
       winner[n] = (sum_m E[n,m] u[m]) / (sum_m E[n,m]) + const,
   u = V c = R (Wv^T c) + bv.c — a per-asset scalar. The V projection and
   attention*V matmul vanish.

2. gamma = Q K^T = R A R^T + (R Wq^T bk)[n] + (R Wk^T bq)[m] + bq.bk with
   A = Wq^T Wk. The per-n term scales E rows uniformly and cancels in the
   s/rowsum ratio, so it is dropped. The per-m term v[m] rides the exp
   activation's per-partition bias slot. The Q and K projections collapse
   into a single projection B = A^T-pack @ R^T.

Per-core device work (batch row b):
  phase A: B[q,m] = sum_q' A[q,q'] R[m,q'] (bf16, qc-outer waves so matmuls
           start when the first R chunk lands); u/v rows as M=1 projections,
           transposed to [128,16] columns via K=1 matmuls against a ones
           scalar.
  phase B: per 128-row m-chunk: gamma^T = B^T-slice @ R^T (PSUM fp32),
           exp(scale*psum + v) -> ET bf16 (ACT), then [u_chunk|ones]^T @ ET
           accumulates s[n] (partition 0) and rowsum[n] (partition 32).
           The s/rowsum matmuls trail one m-chunk behind the score matmuls
           so the PE never waits on exp.
  out: s and rowsum copied to SBUF, DMA'd to DRAM [2, 2048] f32; the host
       does winner = s/rowsum + const.
"""

import math

import ml_dtypes
import numpy as np

import concourse.bass as bass
import concourse.mybir as mybir
import concourse.tile as tile
from concourse.bass_utils import run_bass_kernel_spmd
from concourse.vector_clock import ScopedClock


N_CORES = 8
NB, NN, DD = 8, 2048, 512  # batch, assets, feature dim
P = 128
NQ = DD // P   # q chunks (contraction)
NM = NN // P   # m chunks (key/asset rows)
S = 512        # matmul moving free dim / PSUM bank width
NS = NN // S   # n slices of 512
BF16 = mybir.dt.bfloat16
F32 = mybir.dt.float32
SCALE = 1.0 / math.sqrt(float(DD))
BF = ml_dtypes.bfloat16


class _TileContext(tile.TileContext):
    """Workaround for walrus rejecting >1 sem wait on the kernel-tail Drain
    ("Too many sync wait commands"): put each final wait on its own SP NoOp
    ahead of an unwaited Drain."""

    def _drain_and_barrier(self, tick_clock, wait_clock):
        nc = self.nc
        probe = nc.sync.nop(nofuse=True)
        wait_clock.add_sem_waits(
            probe.ins, ScopedClock({None: tick_clock.global_clock})
        )
        si = probe.ins.sync_info
        waits = list(si.on_wait) if si is not None else []
        if si is not None:
            si.on_wait = []
        # spread the final waits round-robin over all engines so they
        # resolve in parallel; the barrier then guarantees every wait has
        # been observed before the SP drain runs.
        engines = [nc.sync, nc.vector, nc.scalar, nc.tensor, nc.gpsimd]
        for i, w in enumerate(waits):
            n = engines[i % len(engines)].nop(nofuse=True)
            n.ins.sync_info = mybir.SyncInfo(on_wait=[w], on_update=[])
        nc.all_engine_barrier()
        nc.sync.drain()
        assert self.sems is not None
        popped = nc._tile_sem_poison_stack.pop()
        assert popped is self._sem_poison
        # clear_and_free_semaphores would range-clear every ALLOCATED sem id
        # (~200+), which walrus lowers to one op per id (~7us of tail).
        # Only ids that appear in the final instruction stream can be
        # non-zero, so hardware-clear just those; do the allocator
        # bookkeeping for the full set.
        allocated = list(self.sems.allocated().values())
        sem_nums = [
            s.num if hasattr(s, "num") else int(s) for s in allocated
        ]
        used = set()
        for fn in nc.m.functions:
            for blk in fn.blocks:
                for inst in blk.instructions:
                    si = inst.sync_info
                    if si is not None:
                        for w in si.on_wait:
                            used.add(w.id)
                        for u in si.on_update:
                            used.add(u.id)
        hw_nums = sorted(n for n in sem_nums if n in used)
        for sem_range in bass.compact_to_ranges(hw_nums):
            nc.gpsimd.dma_reset(sem_range)
            nc.gpsimd.sem_clear(sem_range)
        nc._state.prepend_free_semaphores(sem_nums)
        for poison_set in nc._tile_sem_poison_stack:
            poison_set.update(sem_nums)
        # the trailing all_engine_barrier is skipped: nothing after the
        # clear touches semaphores, and the runtime serializes executions


def _split_multi_waits(nc, maxw=1):
    """This walrus build rejects instructions carrying more than one sync
    wait ("Too many sync wait commands"). Move excess waits onto same-engine
    NoOps inserted just before the instruction: sem-ge waits are monotonic
    within the kernel, so waiting for them earlier on the same engine is
    equivalent. sem-eq waits stay on the original instruction."""
    for fn in nc.m.functions:
        for blk in fn.blocks:
            insts = blk.instructions
            if not any(
                i.sync_info is not None and len(i.sync_info.on_wait) > maxw
                for i in insts
            ):
                continue
            out = []
            for inst in insts:
                si = inst.sync_info
                if si is not None and len(si.on_wait) > maxw:
                    keep = [w for w in si.on_wait if "eq" in w.wait_mode]
                    movable = [w for w in si.on_wait if "eq" not in w.wait_mode]
                    while len(keep) < maxw and movable:
                        keep.append(movable.pop(0))
                    assert len(keep) <= maxw, (
                        f"{inst.name}: {len(keep)} non-splittable waits"
                    )
                    for w in movable:
                        nop = mybir.InstNoOp(
                            name=nc.get_next_instruction_name(), ins=[], outs=[]
                        )
                        nop.engine = inst.engine
                        nop.sync_info = mybir.SyncInfo(on_wait=[w], on_update=[])
                        out.append(nop)
                    si.on_wait = keep
                out.append(inst)
            blk.instructions = out


def _build():
    nc = bass.Bass("TRN2", target_bir_lowering=False, debug=False)

    rt = nc.dram_tensor("rt", (NQ, P, NN), BF16, kind="ExternalInput")
    amat = nc.dram_tensor("amat", (NQ, P, DD), BF16, kind="ExternalInput")
    wuv = nc.dram_tensor("wuv", (NQ, P, 33), BF16, kind="ExternalInput")
    betas = nc.dram_tensor("betas", (33, 2), F32, kind="ExternalInput")
    out = nc.dram_tensor("out", (2, NN), F32, kind="ExternalOutput")

    Ident = mybir.ActivationFunctionType.Identity
    Copy = mybir.ActivationFunctionType.Copy
    Exp = mybir.ActivationFunctionType.Exp

    with _TileContext(nc) as tc:
        with (
            tc.tile_pool(name="const", bufs=1) as cpool,
            tc.tile_pool(name="big", bufs=1) as big,
            tc.tile_pool(name="et", bufs=4) as et_pool,
            tc.tile_pool(name="dscratch", bufs=1, space="DRAM") as dpool,
        ):
            # rt0 + amat chunks lead on the HWDGE (sync) queue so the first
            # projection wave can start ASAP; rt2/rt3 stream on SWDGE.
            rt_sb = [cpool.tile([P, NN], BF16, name=f"rt{qc}") for qc in range(NQ)]
            a_sb = [cpool.tile([P, DD], BF16, name=f"a{qc}") for qc in range(NQ)]
            # a0 (tiny) first, then rt0 split across both queue types so the
            # first projection wave's critical inputs land earliest
            nc.sync.dma_start(a_sb[0][:], amat.ap()[0])
            nc.sync.dma_start(rt_sb[0][:, : NN // 2], rt.ap()[0][:, : NN // 2])
            nc.gpsimd.dma_start(rt_sb[0][:, NN // 2 :], rt.ap()[0][:, NN // 2 :])
            wuv_sb = cpool.tile([P, NQ, 33], BF16)
            nc.gpsimd.dma_start(wuv_sb[:], wuv.ap().rearrange("q p c -> p q c"))
            betas_sb = cpool.tile([33, 2], F32)
            nc.gpsimd.dma_start(betas_sb[:], betas.ap())
            nc.gpsimd.dma_start(rt_sb[2][:], rt.ap()[2])
            nc.sync.dma_start(rt_sb[1][:], rt.ap()[1])
            nc.sync.dma_start(a_sb[1][:], amat.ap()[1])
            nc.gpsimd.dma_start(rt_sb[3][:], rt.ap()[3])
            nc.sync.dma_start(a_sb[2][:], amat.ap()[2])
            nc.sync.dma_start(a_sb[3][:], amat.ap()[3])

            bt_sb = [big.tile([P, NN], BF16, name=f"bt{qc}") for qc in range(NQ)]
            uvrow_sb = big.tile([33, NN], BF16)
            vcol_sb = big.tile([P, NM], BF16)
            v_sb = big.tile([P, NM], F32)
            # su columns: 0 = u, 32 = ones (s lands on partition 0, rowsum
            # on partition 32 -- both legal base partitions), rest zero.
            su_sb = big.tile([P, NM, 33], BF16)
            nc.vector.memset(su_sb[:], 0.0)
            nc.vector.memset(su_sb[:, :, 32:33], 1.0)

            # One PSUM pool serves projection, u/v and gamma tiles (same
            # tag -> same 4 rotating slots). No pool release between phases
            # means deps are per-slot instead of whole-zone, so phase B's
            # first matmuls don't wait on the entire phase-A cast clock.
            # srs gets the other 4 banks, allocated first and only touched
            # after exp(0).
            psR = tc.alloc_tile_pool(name="psR", bufs=1, space="PSUM")
            psMain = tc.alloc_tile_pool(name="psMain", bufs=4, space="PSUM")
            if True:
                def b_wave(qo):
                    pts = [
                        psMain.tile([P, S], F32, tag="mm", name="mm")
                        for _ in range(NS)
                    ]
                    for qi in range(NQ):
                        for ns in range(NS):
                            nc.tensor.matmul(
                                pts[ns][:],
                                a_sb[qi][:, qo * P : (qo + 1) * P],
                                rt_sb[qi][:, ns * S : (ns + 1) * S],
                                start=(qi == 0),
                                stop=(qi == NQ - 1),
                            )
                    for ns in range(NS):
                        nc.vector.tensor_copy(
                            bt_sb[qo][:, ns * S : (ns + 1) * S],
                            pts[ns][:],
                        )

                def uv_rows():
                    # one M=33 pass computes both u (partition 0) and v
                    # (partition 32, pre-scaled) from the [wtl|w2tl] lhsT
                    for ns in range(NS):
                        pur = psMain.tile([P, S], F32, tag="mm", name="mm")[0:33, :]
                        for qc in range(NQ):
                            nc.tensor.matmul(
                                pur[:],
                                wuv_sb[:, qc, :],
                                rt_sb[qc][:, ns * S : (ns + 1) * S],
                                start=(qc == 0),
                                stop=(qc == NQ - 1),
                            )
                        nc.scalar.activation(
                            uvrow_sb[0:1, ns * S : (ns + 1) * S],
                            pur[0:1, :],
                            Ident,
                            bias=betas_sb[0:1, 0:1],
                            scale=1.0,
                        )
                        nc.scalar.activation(
                            uvrow_sb[32:33, ns * S : (ns + 1) * S],
                            pur[32:33, :],
                            Ident,
                            bias=betas_sb[32:33, 1:2],
                            scale=SCALE,
                        )

                b_wave(0)
                uv_rows()
                b_wave(1)
                b_wave(2)
                b_wave(3)

                # scatter rows [1, 2048] -> columns [128, 16] off the PE:
                # bounce through flat DRAM, where the partition-scatter read
                # pattern is expressible.
                uv_dram = dpool.tile([2, NN], BF16)
                nc.sync.dma_start(uv_dram[0:1, :], uvrow_sb[0:1, :])
                nc.sync.dma_start(uv_dram[1:2, :], uvrow_sb[32:33, :])
                with nc.allow_non_contiguous_dma(
                    reason="2048-elem partition scatter, one-off"
                ):
                    nc.sync.dma_start(
                        su_sb[:, :, 0],
                        uv_dram[0, :].rearrange("(m p) -> p m", p=P),
                    )
                    nc.sync.dma_start(
                        vcol_sb[:],
                        uv_dram[1, :].rearrange("(m p) -> p m", p=P),
                    )
                nc.vector.tensor_copy(v_sb[:], vcol_sb[:])

            # ---- phase B: scores, exp, s/rowsum accumulation ----
            if True:
                srs = [
                    psR.tile([33, S], F32, tag=f"srs{ns}", name=f"srs{ns}")
                    for ns in range(NS)
                ]
                ets = {}

                def gamma(mc):
                    et = et_pool.tile([P, NN], BF16, tag="et", name="et")
                    ets[mc] = et
                    # ns-outer, one PSUM tile in flight at a time: each bank
                    # frees right after its exp, so three slots pipeline
                    # (walrus emits LDWEIGHTS per matmul regardless of loop
                    # order -- ldw-opt is force-disabled -- so the extra
                    # weight reloads here cost nothing extra).
                    for ns in range(NS):
                        g = psMain.tile([P, S], F32, tag="mm", name="mm")
                        for qc in range(NQ):
                            nc.tensor.matmul(
                                g[:],
                                bt_sb[qc][:, mc * P : (mc + 1) * P],
                                rt_sb[qc][:, ns * S : (ns + 1) * S],
                                start=(qc == 0),
                                stop=(qc == NQ - 1),
                            )
                        nc.scalar.activation(
                            et[:, ns * S : (ns + 1) * S],
                            g[:],
                            Exp,
                            bias=v_sb[:, mc : mc + 1],
                            scale=SCALE,
                        )

                def srs_mms(mc):
                    et = ets.pop(mc)
                    for ns in range(NS):
                        nc.tensor.matmul(
                            srs[ns][:],
                            su_sb[:, mc, :],
                            et[:, ns * S : (ns + 1) * S],
                            start=(mc == 0),
                            stop=(mc == NM - 1),
                            skip_group_check=True,
                        )

                # s/rowsum matmuls trail one m-chunk behind the score
                # matmuls so the PE never stalls on the exp activations.
                gamma(0)
                for mc in range(1, NM):
                    gamma(mc)
                    srs_mms(mc - 1)
                srs_mms(NM - 1)

                # copy PSUM -> SBUF (rows 0..32), then DMA rows 0 and 32 out
                out_sb = big.tile([33, NN], F32)
                for ns in range(NS):
                    sl = slice(ns * S, (ns + 1) * S)
                    # alternate DVE/ACT so the four drain copies run on two
                    # engines in parallel
                    if ns % 2 == 0:
                        nc.vector.tensor_copy(out_sb[:, sl], srs[ns][:])
                    else:
                        nc.scalar.copy(out_sb[:, sl], srs[ns][:])
                nc.sync.dma_start(out.ap()[0:1, :], out_sb[0:1, :])
                nc.sync.dma_start(out.ap()[1:2, :], out_sb[32:33, :])
            psMain.release()
            psR.release()

    _split_multi_waits(nc)
    return nc


_NC = None


def _get_nc():
    global _NC
    if _NC is None:
        _NC = _build()
    return _NC


def _pack_pq(a):
    """[512, X] -> [128, 4, X] with (p, chunk) partition striping."""
    return np.ascontiguousarray(a.reshape(4, P, -1).transpose(1, 0, 2))


def kernel(R, Wq, bq, Wk, bk, Wv, bv, W1, b1, W2, b2):
    R = np.asarray(R, np.float32)
    Wq = np.asarray(Wq, np.float64)
    bq = np.asarray(bq, np.float64)
    Wk = np.asarray(Wk, np.float64)
    bk = np.asarray(bk, np.float64)
    Wv = np.asarray(Wv, np.float64)
    bv = np.asarray(bv, np.float64)
    W1 = np.asarray(W1, np.float64)
    b1 = np.asarray(b1, np.float64)
    W2 = np.asarray(W2, np.float64)
    b2 = np.asarray(b2, np.float64)

    # Collapse the linear head: winner = c.a + const, u = V c.
    c = W1.T @ W2[0]                      # [512]
    wtilde = Wv.T @ c                     # [512]
    beta = float(bv @ c)
    const = float(W2[0] @ b1 + b2[0])
    # Collapse the Q/K projections: gamma = R A R^T + v[m] (+ dropped n-term)
    at = Wk.T @ Wq                        # A^T = Wk^T Wq, [q', q]
    w2tilde = Wk.T @ bq                   # [512]
    beta2 = float(bq @ bk)

    a_h = np.ascontiguousarray(at.reshape(4, P, DD)).astype(BF)    # [4,128,512]
    wuv_h = np.zeros((4, P, 33), BF)
    wuv_h[:, :, 0] = wtilde.reshape(4, P).astype(BF)
    wuv_h[:, :, 32] = w2tilde.reshape(4, P).astype(BF)
    betas_h = np.zeros((33, 2), np.float32)
    betas_h[0, 0] = beta
    betas_h[32, 1] = beta2 * SCALE

    in_maps = []
    for b in range(NB):
        # [4, 128, 2048]: chunk-major so each q-chunk is one contiguous DMA
        rt_h = np.ascontiguousarray(R[b].T.reshape(4, P, NN)).astype(BF)
        in_maps.append(
            {
                "rt": rt_h,
                "amat": a_h,
                "wuv": wuv_h,
                "betas": betas_h,
            }
        )

    nc = _get_nc()
    res = run_bass_kernel_spmd(nc, in_maps, core_ids=list(range(N_CORES)))
    outs = np.stack([res.results[b]["out"] for b in range(NB)])   # [8,2,2048]
    return (outs[:, 0] / outs[:, 1] + np.float32(const)).astype(np.float32)



# revision 3
# speedup vs baseline: 1.5973x; 1.5973x over previous
"""CAAN kernel for Trainium2, 8-core data-parallel (one batch row per core).

Math: the reference is
    Q = R Wq^T + bq ; K = R Wk^T + bk ; V = R Wv^T + bv
    E = exp(Q K^T / sqrt(512)) ; saat = E / rowsum(E)
    winner = (saat V) W1^T W2^T + (W2 b1 + b2)

Algebraic collapses (done on host in fp64):

1. The W1/W2 head is linear, so with c = W1^T W2[0]:
       winner[n] = (sum_m E[n,m] u[m]) / (sum_m E[n,m]) + const,
   u = V c = R (Wv^T c) + bv.c — a per-asset scalar.

2. gamma = Q K^T = R A R^T + t[n] + v[m] + bq.bk with A = Wq^T Wk,
   t = R Wq^T bk, v = R Wk^T bq. The per-n term t scales E rows
   uniformly and cancels in the s/rowsum ratio. When bq == 0 (always
   true for this reference), v and bq.bk vanish too, so only
   gamma~ = R A R^T survives on device.

Device layout ("E layout": query index n on partitions, key index m on
the free axis):
  phase A: ct[q, n] = (R A*32)[n, q] via fp8e4 DoubleRow matmuls
           (contraction 256/MM), PSUM fp32 -> fp8 casts alternating
           DVE/ACT.
  phase B: per 128-query chunk: gamma^~ = ct-slice^T R^T into a 4-bank
           [128, 2048] PSUM tile (8 DR matmuls), then ONE activation
           Exp over all 2048 columns -> et bf16, with accum_out
           giving rowsum[n] for free; one DVE tensor_tensor_reduce
           (et * u_bcast) gives s[n]. No PE work for the reductions.
  out: s and rowsum columns [128, 16] f32 DMA'd out; host does
       winner = s/rowsum + const.

fp8 notes: A is pre-scaled by 32 so its entries (std ~0.016) clear the
e4m3 denormal floor; the inverse rides the exp scale. Validated on CPU:
rel err ~2e-3 vs the fp32 reference (tolerance 2e-2).
"""

import math

import ml_dtypes
import numpy as np

import concourse.bass as bass
import concourse.mybir as mybir
import concourse.tile as tile
from concourse.bass_utils import run_bass_kernel_spmd
from concourse.vector_clock import ScopedClock


N_CORES = 8
NB, NN, DD = 8, 2048, 512  # batch, assets, feature dim
P = 128
NQ = DD // P   # q chunks (contraction)
NC = NN // P   # n chunks (query rows)
S = 512        # matmul moving free dim / PSUM bank width
NS = NN // S   # slices of 512 along the free axis
BF16 = mybir.dt.bfloat16
FP8 = mybir.dt.float8e4
F32 = mybir.dt.float32
SCALE = 1.0 / math.sqrt(float(DD))
ASCALE = 32.0
BF = ml_dtypes.bfloat16
F8 = ml_dtypes.float8_e4m3
DR = mybir.MatmulPerfMode.DoubleRow


class _TileContext(tile.TileContext):
    """Workaround for walrus rejecting >1 sem wait on the kernel-tail Drain
    ("Too many sync wait commands"): put each final wait on its own SP NoOp
    ahead of an unwaited Drain."""

    def _drain_and_barrier(self, tick_clock, wait_clock):
        nc = self.nc
        probe = nc.sync.nop(nofuse=True)
        wait_clock.add_sem_waits(
            probe.ins, ScopedClock({None: tick_clock.global_clock})
        )
        si = probe.ins.sync_info
        waits = list(si.on_wait) if si is not None else []
        if si is not None:
            si.on_wait = []
        # spread the final waits round-robin over all engines so they
        # resolve in parallel; the barrier then guarantees every wait has
        # been observed before the SP drain runs.
        engines = [nc.sync, nc.vector, nc.scalar, nc.tensor, nc.gpsimd]
        for i, w in enumerate(waits):
            n = engines[i % len(engines)].nop(nofuse=True)
            n.ins.sync_info = mybir.SyncInfo(on_wait=[w], on_update=[])
        nc.all_engine_barrier()
        nc.sync.drain()
        assert self.sems is not None
        popped = nc._tile_sem_poison_stack.pop()
        assert popped is self._sem_poison
        # clear_and_free_semaphores would range-clear every ALLOCATED sem id
        # (~200+), which walrus lowers to one op per id (~7us of tail).
        # Only ids that appear in the final instruction stream can be
        # non-zero, so hardware-clear just those; do the allocator
        # bookkeeping for the full set.
        allocated = list(self.sems.allocated().values())
        sem_nums = [
            s.num if hasattr(s, "num") else int(s) for s in allocated
        ]
        used = set()
        for fn in nc.m.functions:
            for blk in fn.blocks:
                for inst in blk.instructions:
                    si = inst.sync_info
                    if si is not None:
                        for w in si.on_wait:
                            used.add(w.id)
                        for u in si.on_update:
                            used.add(u.id)
        hw_nums = sorted(n for n in sem_nums if n in used)
        for sem_range in bass.compact_to_ranges(hw_nums):
            nc.gpsimd.dma_reset(sem_range)
            nc.gpsimd.sem_clear(sem_range)
        nc._state.prepend_free_semaphores(sem_nums)
        for poison_set in nc._tile_sem_poison_stack:
            poison_set.update(sem_nums)
        # the trailing all_engine_barrier is skipped: nothing after the
        # clear touches semaphores, and the runtime serializes executions


def _split_multi_waits(nc, maxw=1):
    """This walrus build rejects instructions carrying more than one sync
    wait ("Too many sync wait commands"). Move excess waits onto same-engine
    NoOps inserted just before the instruction: sem-ge waits are monotonic
    within the kernel, so waiting for them earlier on the same engine is
    equivalent. sem-eq waits stay on the original instruction."""
    for fn in nc.m.functions:
        for blk in fn.blocks:
            insts = blk.instructions
            if not any(
                i.sync_info is not None and len(i.sync_info.on_wait) > maxw
                for i in insts
            ):
                continue
            out = []
            for inst in insts:
                si = inst.sync_info
                if si is not None and len(si.on_wait) > maxw:
                    keep = [w for w in si.on_wait if "eq" in w.wait_mode]
                    movable = [w for w in si.on_wait if "eq" not in w.wait_mode]
                    while len(keep) < maxw and movable:
                        keep.append(movable.pop(0))
                    assert len(keep) <= maxw, (
                        f"{inst.name}: {len(keep)} non-splittable waits"
                    )
                    for w in movable:
                        nop = mybir.InstNoOp(
                            name=nc.get_next_instruction_name(), ins=[], outs=[]
                        )
                        nop.engine = inst.engine
                        nop.sync_info = mybir.SyncInfo(on_wait=[w], on_update=[])
                        out.append(nop)
                    si.on_wait = keep
                out.append(inst)
            blk.instructions = out


def _build(general: bool):
    """general=False assumes bq == 0 (rowsum = plain sum of exp, via the
    activation's accum_out). general=True computes both reductions with
    explicit per-key weight rows (w1 = phi*u, w0 = phi, phi = exp(v*SCALE))
    so arbitrary biases still work."""
    nc = bass.Bass("TRN2", target_bir_lowering=False, debug=False)

    rt = nc.dram_tensor("rt", (P, NQ, NN), FP8, kind="ExternalInput")
    am = nc.dram_tensor("am", (P, NQ, DD), FP8, kind="ExternalInput")
    ub = nc.dram_tensor("ub", (P, NN), BF16, kind="ExternalInput")
    if general:
        wb = nc.dram_tensor("wb", (P, NN), BF16, kind="ExternalInput")
    out = nc.dram_tensor("out", (2, P, NC), F32, kind="ExternalOutput")

    Exp = mybir.ActivationFunctionType.Exp
    Mult = mybir.AluOpType.mult
    Add = mybir.AluOpType.add

    with _TileContext(nc) as tc:
        with (
            tc.tile_pool(name="const", bufs=1) as cpool,
            tc.tile_pool(name="et", bufs=3) as et_pool,
        ):
            # ACT exp-table pre-warm: a [1,1] exp at t=0 so the ~2.7us
            # table load overlaps the input DMA instead of stalling the
            # first real exp.
            warm = cpool.tile([1, 1], F32)
            nc.vector.memset(warm[:], 0.0)
            nc.scalar.activation(warm[:], warm[:], Exp, bias=0.0, scale=0.0)

            rt_sb = cpool.tile([P, NQ, NN], FP8, name="rt")
            am_sb = cpool.tile([P, NQ, DD], FP8, name="am")
            ub_sb = cpool.tile([P, NN], BF16, name="ub")
            wb_sb = cpool.tile([P, NN], BF16, name="wb") if general else None
            ct_sb = cpool.tile([P, NQ, NN], FP8, name="ct")
            s_cols = cpool.tile([P, NC], F32, name="scols")
            rs_cols = cpool.tile([P, NC], F32, name="rscols")

            # qc 0+1 first (phase A's first DR pair), am split across both
            # queue types so the first matmul's inputs land earliest.
            nc.sync.dma_start(am_sb[:, 0:2, :], am.ap()[:, 0:2, :])
            nc.gpsimd.dma_start(am_sb[:, 2:4, :], am.ap()[:, 2:4, :])
            nc.sync.dma_start(rt_sb[:, 0:1, :], rt.ap()[:, 0:1, :])
            nc.gpsimd.dma_start(rt_sb[:, 1:2, :], rt.ap()[:, 1:2, :])
            nc.sync.dma_start(rt_sb[:, 2:3, :], rt.ap()[:, 2:3, :])
            nc.gpsimd.dma_start(rt_sb[:, 3:4, :], rt.ap()[:, 3:4, :])
            nc.gpsimd.dma_start(ub_sb[:], ub.ap())
            if general:
                nc.gpsimd.dma_start(wb_sb[:], wb.ap())

            ps = tc.alloc_tile_pool(name="ps", bufs=2, space="PSUM")

            # ---- phase A: ct[q, n] = (R A*32)[n, q] in fp8 ----
            for qo in range(NQ):
                pt = ps.tile([P, NN], F32, tag="g", name="g")
                for j in range(NQ // 2):
                    for ns in range(NS):
                        nc.tensor.matmul(
                            pt[:, ns * S : (ns + 1) * S],
                            am_sb[:, 2 * j : 2 * j + 2, qo * P : (qo + 1) * P],
                            rt_sb[:, 2 * j : 2 * j + 2, ns * S : (ns + 1) * S],
                            start=(j == 0),
                            stop=(j == NQ // 2 - 1),
                            perf_mode=DR,
                        )
                # split each fp32->fp8 cast across DVE and ACT so the
                # last cast finishes ~1us after the last phase-A matmul
                half = NN // 2
                nc.vector.tensor_copy(ct_sb[:, qo, 0:half], pt[:, 0:half])
                nc.scalar.copy(ct_sb[:, qo, half:NN], pt[:, half:NN])

            # ---- phase B: exp + reductions per 128-query chunk ----
            for nch in range(NC):
                gt = ps.tile([P, NN], F32, tag="g", name="g")
                for j in range(NQ // 2):
                    for ms in range(NS):
                        nc.tensor.matmul(
                            gt[:, ms * S : (ms + 1) * S],
                            ct_sb[:, 2 * j : 2 * j + 2, nch * P : (nch + 1) * P],
                            rt_sb[:, 2 * j : 2 * j + 2, ms * S : (ms + 1) * S],
                            start=(j == 0),
                            stop=(j == NQ // 2 - 1),
                            perf_mode=DR,
                        )
                et = et_pool.tile([P, NN], BF16, tag="et", name="et")
                nc.scalar.activation(
                    et[:],
                    gt[:],
                    Exp,
                    bias=0.0,
                    scale=SCALE / ASCALE,
                    accum_out=rs_cols[:, nch : nch + 1],
                )
                # fused multiply+free-axis-sum on DVE:
                #   out = (et * 1.0) * ub ; accum = sum(out)
                if general:
                    et2 = et_pool.tile([P, NN], BF16, tag="et2", name="et2")
                    nc.vector.scalar_tensor_tensor(
                        out=et2[:],
                        in0=et[:],
                        scalar=1.0,
                        in1=ub_sb[:],
                        op0=Mult,
                        op1=Mult,
                        accum_out=s_cols[:, nch : nch + 1],
                    )
                    nc.vector.scalar_tensor_tensor(
                        out=et[:],
                        in0=et[:],
                        scalar=1.0,
                        in1=wb_sb[:],
                        op0=Mult,
                        op1=Mult,
                        accum_out=rs_cols[:, nch : nch + 1],
                    )
                else:
                    nc.vector.scalar_tensor_tensor(
                        out=et[:],
                        in0=et[:],
                        scalar=1.0,
                        in1=ub_sb[:],
                        op0=Mult,
                        op1=Mult,
                        accum_out=s_cols[:, nch : nch + 1],
                    )

            nc.sync.dma_start(out.ap()[0], s_cols[:])
            nc.sync.dma_start(out.ap()[1], rs_cols[:])
            ps.release()

    _split_multi_waits(nc)
    return nc


_NC = {}


def _get_nc(general: bool):
    if general not in _NC:
        _NC[general] = _build(general)
    return _NC[general]


def _host_prep(R, Wq, bq, Wk, bk, Wv, bv, W1, b1, W2, b2):
    """Host-side collapses in fp64. Returns (general, per-core input maps,
    const)."""
    c = W1.T @ W2[0]                       # [512]
    const = float(W2[0] @ b1 + b2[0])
    A = Wq.T @ Wk                          # gamma~ = R A R^T
    general = bool(np.any(bq != 0.0))

    a_h = np.ascontiguousarray(
        (A * ASCALE).reshape(NQ, P, DD).transpose(1, 0, 2)
    ).astype(F8)                           # [128, 4, 512]

    in_maps = []
    for b in range(NB):
        Rb = R[b].astype(np.float64)
        rt_h = np.ascontiguousarray(
            Rb.T.reshape(NQ, P, NN).transpose(1, 0, 2)
        ).astype(F8)                       # [128, 4, 2048]
        u = Rb @ (Wv.T @ c) + float(bv @ c)            # [2048]
        m = {"rt": rt_h, "am": a_h}
        if general:
            phi = np.exp((Rb @ (Wk.T @ bq)) * SCALE)   # per-key weight
            m["ub"] = np.ascontiguousarray(
                np.broadcast_to((phi * u).astype(BF), (P, NN))
            )
            m["wb"] = np.ascontiguousarray(
                np.broadcast_to(phi.astype(BF), (P, NN))
            )
        else:
            m["ub"] = np.ascontiguousarray(
                np.broadcast_to(u.astype(BF), (P, NN))
            )
        in_maps.append(m)
    return general, in_maps, const


def kernel(R, Wq, bq, Wk, bk, Wv, bv, W1, b1, W2, b2):
    R = np.asarray(R, np.float32)
    args = [np.asarray(x, np.float64) for x in (Wq, bq, Wk, bk, Wv, bv, W1, b1, W2, b2)]
    general, in_maps, const = _host_prep(R, *args)

    nc = _get_nc(general)
    res = run_bass_kernel_spmd(nc, in_maps, core_ids=list(range(N_CORES)))
    outs = np.stack([res.results[b]["out"] for b in range(NB)])  # [8,2,128,16]
    s = outs[:, 0].transpose(0, 2, 1).reshape(NB, NN)   # n = nch*128 + p
    r = outs[:, 1].transpose(0, 2, 1).reshape(NB, NN)
    return (s / r + np.float32(const)).astype(np.float32)
